# revision 41
# baseline (speedup 1.0000x reference)
"""Multi-head causal attention (B=2, S=2048, D=1024, H=16) on 8 NeuronCores.

Sharding: tensor-parallel over heads (2 heads/core, both batches on every
core). Each core computes q/k/v projections for its 2 heads, causal
attention, and a partial output projection (its 128 rows of W_proj); the
host sums the 8 partials and adds b_proj.

Device-side design (all matmuls bf16 with fp32 PSUM accumulate):
 - x arrives pre-transposed AND pre-tiled from the host
   (xT [128, 8, 4096] = partition-major) so every DMA is contiguous and
   every matmul has its contraction dim on partitions with zero on-chip
   transposes of x.
 - q is produced transposed ([2*64 head-dims, 4096 tokens]); k likewise
   but stored as TWO zero-padded copies (head h in rows 64h..64h+63,
   zeros elsewhere) so each head's score matmul contracts K=128 at the
   full-array rate (K=64 matmuls measure ~2.5x slower on TRN2).
 - scores are computed as ST = K @ Q^T ([keys, queries], 512-wide query
   groups); no row-max is needed (scaled scores are O(6)); one Exp
   instruction per kj-pair psum tile.
 - v is produced token-major [tokens, 130] = [V_h0 | 1 | V_h1 | 1]; the
   bias/ones columns are added via a partition-broadcast vector add at
   copyback. The AV product expST.T @ [V|1] yields context AND the
   softmax denominator in one accumulation group with queries on PSUM
   partitions -> normalization is a per-partition tensor_scalar multiply.
 - causal mask: multiply exp(scores) of the diagonal 128x128 block by a
   0/1 triangle (exact: masked entries contribute exactly 0, matching
   exp(-1e9/8) == 0 in fp32).
 - phases are software-pipelined in program order (projection column
   tiles, attention query groups, output-projection groups) so Tile's
   static list scheduler overlaps PE projection work with the
   ScalarE-bound exp stream and the output projection.
"""

import sys

sys.path.insert(0, "/opt/trn_rl_repo")

import numpy as np
import ml_dtypes

import concourse.bass as bass
import concourse.mybir as mybir
import concourse.tile as tile
from concourse import bacc
from concourse.bass_utils import run_bass_kernel_spmd

BF16 = mybir.dt.bfloat16
F32 = mybir.dt.float32
NPBF16 = ml_dtypes.bfloat16

B, S, D = 2, 2048, 1024
H, DH = 16, 64
T = B * S            # 4096 tokens
KS = D // 128        # 8 contraction subtiles
QT = S // 128        # 16 query tiles per batch
ACT_F = mybir.ActivationFunctionType


def _build_nc():
    # Bacc (not raw Bass): its compile() pass pipeline splits multi-sem
    # waits down to the TRN2 1-wait-per-instruction hardware limit.
    nc = bacc.Bacc("TRN2", target_bir_lowering=False, debug=False, num_devices=8)

    xT = nc.dram_tensor("xT", [128, KS, T], BF16, kind="ExternalInput")
    wq = nc.dram_tensor("wq", [128, KS, 128], BF16, kind="ExternalInput")
    wk = nc.dram_tensor("wk", [128, KS, 128], BF16, kind="ExternalInput")
    wv = nc.dram_tensor("wv", [128, KS, 130], BF16, kind="ExternalInput")
    bq = nc.dram_tensor("bq", [128, 1], F32, kind="ExternalInput")
    bk = nc.dram_tensor("bk", [128, 1], F32, kind="ExternalInput")
    bv = nc.dram_tensor("bv", [1, 130], BF16, kind="ExternalInput")
    wp = nc.dram_tensor("wp", [128, D], BF16, kind="ExternalInput")
    tri = nc.dram_tensor("tri", [128, 128], BF16, kind="ExternalInput")
    out = nc.dram_tensor("o", [T, D], BF16, kind="ExternalOutput")

    with tile.TileContext(nc) as tc:
        with (
            tc.tile_pool(name="singles", bufs=1) as singles,
            # one psum pool: tag "qk" [128,2,512] f32 = 2 banks x 2 bufs,
            # tag "av" [128,65] = 1 bank x 2, tag "po" [128,512] = 1 bank
            # x 2 -> exactly 8 banks
            tc.tile_pool(name="qkps", bufs=2, space="PSUM") as qkps,
            tc.tile_pool(name="expp", bufs=20) as expp,
            tc.tile_pool(name="ctxp", bufs=8) as ctxp,
            tc.tile_pool(name="outp", bufs=2) as outp,
            tc.tile_pool(name="rdp", bufs=4) as rdp,
        ):
            # ---- resident tensors -------------------------------------
            wq_sb = singles.tile([128, KS, 128], BF16, tag="wq")
            wk_sb = singles.tile([128, KS, 128], BF16, tag="wk")
            wv_sb = singles.tile([128, KS, 130], BF16, tag="wv")
            bq_sb = singles.tile([128, 1], F32, tag="bq")
            bk_sb = singles.tile([128, 1], F32, tag="bk")
            # b_v (+ the ones columns) broadcast to all partitions; fused
            # into the v copyback as a tensor_tensor add
            bv_sb = singles.tile([128, 130], BF16, tag="bv")
            wp_sb = singles.tile([128, D], BF16, tag="wp")
            tri_sb = singles.tile([128, 128], BF16, tag="tri")
            xT_sb = singles.tile([128, KS, T], BF16, tag="xT")
            qT_sb = singles.tile([128, T], BF16, tag="qT")
            # kT stored twice, zero-padded per head (see module docstring)
            kTz = [
                singles.tile([128, T], BF16, tag=f"kTz{h}", name=f"kTz{h}")
                for h in range(2)
            ]
            # v, per (batch, key-tile): [V_h0 | 1 | V_h1 | 1]
            v_sb = singles.tile([128, B, QT, 130], BF16, tag="v")
            ctxT_sb = singles.tile([128, T // 128, 128], BF16, tag="ctxT")

            nc.vector.memset(kTz[0][64:128, :], 0.0)
            nc.vector.memset(kTz[1][0:64, :], 0.0)

            nc.sync.dma_start(wq_sb[:], wq[:])
            nc.sync.dma_start(wk_sb[:], wk[:])
            nc.sync.dma_start(
                xT_sb[:, :, bass.ds(0, 512)], xT[:, :, bass.ds(0, 512)]
            )
            nc.sync.dma_start(
                xT_sb[:, :, bass.ds(512, 512)], xT[:, :, bass.ds(512, 512)]
            )
            nc.sync.dma_start(wv_sb[:], wv[:])
            nc.sync.dma_start(
                xT_sb[:, :, bass.ds(1024, 512)], xT[:, :, bass.ds(1024, 512)]
            )
            nc.sync.dma_start(bq_sb[:], bq[:])
            nc.sync.dma_start(bk_sb[:], bk[:])
            nc.sync.dma_start(bv_sb[:], bv[:].to_broadcast((128, 130)))
            nc.sync.dma_start(wp_sb[:], wp[:])
            nc.sync.dma_start(tri_sb[:], tri[:])

            # ---- phase emitters ---------------------------------------
            def emit_proj_tcol(tc8):
                """Projections for one 512-token column tile.

                Uses the "po" psum tag (shared with the late-running
                output projection) so the attention pipeline owns the
                "qk" slots and can start while projections continue.
                """
                csl = bass.ds(tc8 * 512, 512)
                if tc8 + 3 < 8:  # prefetch three column tiles ahead
                    nsl = bass.ds((tc8 + 3) * 512, 512)
                    nc.sync.dma_start(xT_sb[:, :, nsl], xT[:, :, nsl])

                ps_q = qkps.tile([128, 512], F32, tag="po", name="ps_q", bufs=2)
                for ks in range(KS):
                    nc.tensor.matmul(
                        ps_q[:],
                        wq_sb[:, ks, :],
                        xT_sb[:, ks, csl],
                        start=(ks == 0),
                        stop=(ks == KS - 1),
                    )
                nc.scalar.activation(
                    qT_sb[:, csl], ps_q[:], ACT_F.Identity, bias=bq_sb[:], scale=1.0
                )

                ps_k = qkps.tile([128, 512], F32, tag="po", name="ps_k", bufs=2)
                for ks in range(KS):
                    nc.tensor.matmul(
                        ps_k[:],
                        wk_sb[:, ks, :],
                        xT_sb[:, ks, csl],
                        start=(ks == 0),
                        stop=(ks == KS - 1),
                    )
                nc.scalar.activation(
                    kTz[0][0:64, csl],
                    ps_k[0:64, :],
                    ACT_F.Identity,
                    bias=bk_sb[0:64],
                    scale=1.0,
                )
                nc.scalar.activation(
                    kTz[1][64:128, csl],
                    ps_k[64:128, :],
                    ACT_F.Identity,
                    bias=bk_sb[64:128],
                    scale=1.0,
                )

                for jj in range(4):  # v tiles, one 128-token tile each
                    tt = tc8 * 4 + jj
                    vb, vk = divmod(tt, QT)
                    ps_v = qkps.tile(
                        [128, 512], F32, tag="po", name="ps_v", bufs=2
                    )
                    for ks in range(KS):
                        nc.tensor.matmul(
                            ps_v[:, :130],
                            xT_sb[:, ks, bass.ds(tt * 128, 128)],
                            wv_sb[:, ks, :],
                            start=(ks == 0),
                            stop=(ks == KS - 1),
                        )
                    # bias add also writes the ones columns (64, 129)
                    nc.vector.tensor_add(
                        v_sb[:, vb, vk, :], ps_v[:, :130], bv_sb[:]
                    )

            def emit_attn_group(bb, g):
                """Causal attention for one 4-query-tile group."""
                boff = bb * S
                nkj = 4 * g + 4  # kj blocks this group needs
                gsl = bass.ds(boff + g * 512, 512)
                ctxs = [
                    ctxp.tile([128, 128], BF16, tag="ctx", name=f"ctx_{r}")
                    for r in range(4)
                ]
                ex_tiles = [[], []]  # per head
                for j in range(0, nkj, 2):  # kj pairs
                    qks = [
                        qkps.tile([128, 2, 512], F32, tag="qk", name=f"qk_h{h}")
                        for h in range(2)
                    ]
                    for i2 in range(2):
                        kj = j + i2
                        ksl = bass.ds(boff + kj * 128, 128)
                        # queries strictly below kj contribute nothing
                        ri = max(0, kj - 4 * g)
                        qsl = bass.ds(boff + g * 512 + ri * 128, 512 - ri * 128)
                        for h in range(2):
                            nc.tensor.matmul(
                                qks[h][:, i2, bass.ds(ri * 128, 512 - ri * 128)],
                                kTz[h][:, ksl],
                                qT_sb[:, qsl],
                                start=True,
                                stop=True,
                            )
                    # queries below kj are fully masked; skip them
                    rlo = max(0, j - 4 * g)
                    esl = bass.ds(rlo * 128, 512 - rlo * 128)
                    for h in range(2):
                        ex = expp.tile([128, 2, 512], BF16, tag="exp")
                        nc.scalar.activation(
                            ex[:, :, esl], qks[h][:, :, esl], ACT_F.Exp, scale=0.125
                        )
                        ex_tiles[h].append(ex)
                for h in range(2):
                    for r in range(4):  # zero masked triangle on diagonal
                        qi = 4 * g + r
                        dsl = bass.ds(r * 128, 128)
                        exd = ex_tiles[h][qi // 2]
                        nc.vector.tensor_mul(
                            exd[:, qi % 2, dsl], exd[:, qi % 2, dsl], tri_sb[:]
                        )
                for h in range(2):
                    hsl = slice(64 * h, 64 * h + 64)
                    for r in range(4):
                        qi = 4 * g + r
                        av = qkps.tile([128, 65], F32, tag="av", bufs=2)
                        for kj in range(qi + 1):
                            nc.tensor.matmul(
                                av[:],
                                ex_tiles[h][kj // 2][:, kj % 2, bass.ds(r * 128, 128)],
                                v_sb[:, bb, kj, bass.ds(65 * h, 65)],
                                start=(kj == 0),
                                stop=(kj == qi),
                            )
                        rd = rdp.tile([128, 1], F32, tag="rd")
                        nc.vector.reciprocal(rd[:], av[:, 64:65])
                        nc.vector.tensor_scalar_mul(
                            ctxs[r][:, hsl], av[:, 0:64], rd[:]
                        )
                for r in range(4):
                    nc.sync.dma_start(
                        ctxT_sb[:, bb * QT + 4 * g + r, :], ctxs[r][:], transpose=True
                    )

            out_r = out.rearrange("(n p) d -> p n d", p=128)

            def emit_out_group(i, act_share=False):
                """Output projection for the 4 token tiles of group i."""
                ot = outp.tile([128, 4, D], BF16, tag="out", name="ot")
                for j4 in range(4):
                    tt = i * 4 + j4
                    for half in range(2):
                        po = qkps.tile([128, 512], F32, tag="po", name="ps_o", bufs=2)
                        nc.tensor.matmul(
                            po[:],
                            ctxT_sb[:, tt, :],
                            wp_sb[:, bass.ds(half * 512, 512)],
                            start=True,
                            stop=True,
                        )
                        osl = bass.ds(half * 512, 512)
                        if act_share and half == 1:
                            nc.scalar.copy(ot[:, j4, osl], po[:])
                        else:
                            nc.vector.tensor_copy(ot[:, j4, osl], po[:])
                nc.gpsimd.dma_start(out_r[:, bass.ds(i * 4, 4), :], ot[:])

            # ---- schedule ---------------------------------------------
            # Attention group (bb, g) needs projection column tiles
            # bb*4 .. bb*4+g. Interleave so attention (which owns the
            # "qk" psum slots) starts as early as possible; output
            # projection follows each attention group with a lag of 1.
            # b1's groups run 1,2,3,0 so a small group (few exps) is
            # last, shortening the drain tail.
            sched = [
                ("P", 0), ("P", 1),
                ("A", 0, 0), ("P", 2),
                ("A", 0, 1), ("P", 3),
                ("A", 0, 2), ("P", 4),
                ("A", 0, 3), ("P", 5),
                ("A", 1, 1), ("P", 6),
                ("A", 1, 2), ("P", 7),
                ("A", 1, 3),
                ("A", 1, 0),
            ]
            prev_a = None
            for step in sched:
                if step[0] == "P":
                    emit_proj_tcol(step[1])
                else:
                    _, bb, g = step
                    emit_attn_group(bb, g)
                    if prev_a is not None:
                        last = prev_a == (1, 3)
                        emit_out_group(prev_a[0] * 4 + prev_a[1], act_share=last)
                    prev_a = (bb, g)
            emit_out_group(prev_a[0] * 4 + prev_a[1], act_share=True)

    return nc


_NC_CACHE = None


def _get_nc():
    global _NC_CACHE
    if _NC_CACHE is None:
        nc = _build_nc()
        nc.finalize()  # runs Bacc's pass pipeline (sync-wait splitting etc.)
        _NC_CACHE = nc
    return _NC_CACHE


def _make_in_maps(x, W_qkv, b_qkv, W_proj):
    # x [B,S,D] -> xT tiled [128 partitions, KS, T] (contraction-major)
    xT = np.ascontiguousarray(
        x.reshape(T, D).astype(NPBF16).reshape(T, KS, 128).transpose(2, 1, 0)
    )
    tri = np.triu(np.ones((128, 128), dtype=np.float32)).astype(NPBF16)

    def wtile(w):  # [D, M] -> [128, KS, M] contraction-major tiles
        m = w.shape[1]
        return np.ascontiguousarray(
            w.astype(NPBF16).reshape(KS, 128, m).transpose(1, 0, 2)
        )

    in_maps = []
    for c in range(8):
        cs = slice(128 * c, 128 * c + 128)
        wq = wtile(W_qkv[:, 0 * D :][:, cs])
        wk = wtile(W_qkv[:, 1 * D :][:, cs])
        v_blk = W_qkv[:, 2 * D :][:, cs].astype(np.float32)
        wv = np.zeros((D, 130), dtype=np.float32)
        wv[:, 0:64] = v_blk[:, 0:64]
        wv[:, 65:129] = v_blk[:, 64:128]
        bvv = np.zeros((1, 130), dtype=np.float32)
        bvv[0, 0:64] = b_qkv[2 * D :][cs][0:64]
        bvv[0, 65:129] = b_qkv[2 * D :][cs][64:128]
        bvv[0, 64] = 1.0
        bvv[0, 129] = 1.0
        in_maps.append(
            {
                "xT": xT,
                "wq": wq,
                "wk": wk,
                "wv": wtile(wv),
                "bq": np.ascontiguousarray(
                    b_qkv[0 * D :][cs].astype(np.float32).reshape(128, 1)
                ),
                "bk": np.ascontiguousarray(
                    b_qkv[1 * D :][cs].astype(np.float32).reshape(128, 1)
                ),
                "bv": bvv.astype(NPBF16),
                "wp": np.ascontiguousarray(W_proj[cs, :].astype(NPBF16)),
                "tri": tri,
            }
        )
    return in_maps


def kernel(x, W_qkv, b_qkv, W_proj, b_proj, **run_kwargs):
    x = np.asarray(x, dtype=np.float32)
    W_qkv = np.asarray(W_qkv, dtype=np.float32)
    b_qkv = np.asarray(b_qkv, dtype=np.float32)
    W_proj = np.asarray(W_proj, dtype=np.float32)
    b_proj = np.asarray(b_proj, dtype=np.float32)

    nc = _get_nc()
    in_maps = _make_in_maps(x, W_qkv, b_qkv, W_proj)
    res = run_bass_kernel_spmd(nc, in_maps, core_ids=list(range(8)), **run_kwargs)

    acc = np.zeros((T, D), dtype=np.float32)
    for c in range(8):
        acc += res.results[c]["o"].astype(np.float32)
    acc += b_proj[None, :]
    out = acc.reshape(B, S, D)
    kernel.last_result = res
    return out
